# revision 65
# baseline (speedup 1.0000x reference)
"""Multi-head attention Trainium2 Bass kernel.

Problem: B=8, S=1024, D=768, H=12, head_dim=64; per-head block-diagonal QKV
projections + softmax attention (no 1/sqrt(hd) scaling).

Sharding: data-parallel over batch - one batch element per NeuronCore (8
cores). No collectives; host scatters inputs / gathers outputs.

Host-side prep (free - not on the HW clock): x is pre-transposed to
xT [D, S] (f32 and bf16 copies), and the per-head weight stacks are packed
into block-diagonal pair matrices so each head-pair's QKV projection is a
single 128-contraction matmul:
  wqk [128, 6, 2, 128]  blockdiag(W[2p], W[2p+1]) for q (j=0) / k (j=1)
  bqk [128, 6, 2]       per-partition bias columns
  wv2 [128, 6, 130]     blockdiag Wv pair, 65-wide halves; col 64/129 = 0
  bv2 [1, 6, 130]       bv pair with 1.0 in cols 64/129 (softmax denom trick)

Per-core dataflow (channel-on-partition layouts; head pairs p = (2p, 2p+1)
match 128-row blocks of xT):
  qT/kT[p] = wqk[p].T @ xT[p] + bqk   (f32r, one [128,512] matmul per half)
  v~[p]    = xT_bf[p].T @ wv2[p] (+ rank-1 bv2)   [t, tt, 130] bf16
  LT       = K Q^T  in 64 psum tiles [128, 3, 512] (3 banks each, bufs=2)
  E^T      = exp(LT)  one ScalarE op per 1536-wide tile (the bottleneck:
             64 x (1536+222) cycles at 1.2 GHz ~= 94 us busy)
  O        = E @ v~ per (pair, s-tile): [128, 2, 65]-shaped psum groups;
             col 64 of each head half = softmax denominator (ones column)
  out      = O * recip(denom) -> staging [128, 8, 768] -> chunked DMAs

Scheduling: ScalarE exp is a solid 64-tile ladder; everything else rides it
as fillers keyed by exp-tile index (see the `plan` dict): projections and
v-projections sit in AV-free tile windows, AV groups follow their pair's
last exp, and writeback DMAs are chunked per s-tile column block. PSUM
banks: lt 2x3 + po 1 + scr 1 = 8; sequential-only accumulation groups per
bank region (interleaved groups in one bank corrupt - hw start-flag
semantics). The lead-in hides the PE p-state ramp behind junk matmuls and
orders DMAs/bias-adds so the first exp fires at ~6.6 us; the tail drains
pair-5 head-1 with a 4-deep psum rotation, ScalarE-assisted normalizes,
and small trailing DMAs.
"""
import numpy as np

S = 1024
D = 768
H = 12
HD = 64
NPAIR = H // 2   # 6
NCORES = 8
ST = S // 128    # 8 s-tiles
TT = S // 128    # 8 t-tiles
NCHUNK = 192     # 512-col logit chunks
NTILE = 64       # exp tiles (3 chunks each)

_CACHE = {}


def _chunk_order():
    """Flat emission order of 512-col logit chunks: pairs sequential. Pair 0
    is grouped by (kT s-half = tt//4, qT s-half) so the first exp tiles only
    need the first q/k bias-add of the projection; middle pairs are tt-major;
    the last pair is hh-major so head-0's AV can drain while head-1's exps
    still run."""
    order = []
    for khalf in (0, 1):
        for half in (0, 1):
            for tt in range(4 * khalf, 4 * khalf + 4):
                for hh in range(2):
                    order.append((0, tt, hh, half))
    for p in range(1, NPAIR - 1):
        for tt in range(TT):
            for hh in range(2):
                order.append((p, tt, hh, 0))
                order.append((p, tt, hh, 1))
    p = NPAIR - 1
    for hh in range(2):
        for tt in range(TT):
            order.append((p, tt, hh, 0))
            order.append((p, tt, hh, 1))
    return order


_CHUNKS = _chunk_order()
_CHUNK_IDX = {key: c for c, key in enumerate(_CHUNKS)}


def _et_loc(p, tt, hh, st):
    """(exp-tile index, col offset) of the [128,128] E^T slice for s-tile st
    of unit (p, tt, hh)."""
    c = _CHUNK_IDX[(p, tt, hh, 0 if st < 4 else 1)]
    return c // 3, (c % 3) * 512 + (st % 4) * 128


def _build():
    import contextlib
    import concourse.bacc as bacc
    import concourse.mybir as mybir
    import concourse.tile as tile

    f32 = mybir.dt.float32
    f32r = mybir.dt.float32r
    bf16 = mybir.dt.bfloat16
    Exp = mybir.ActivationFunctionType.Exp

    nc = bacc.Bacc("TRN2", target_bir_lowering=False, debug=False,
                   num_devices=NCORES)
    xt = nc.declare_dram_parameter("xt", [D, S], f32, isOutput=False)
    xtb = nc.declare_dram_parameter("xtb", [D, S], bf16, isOutput=False)
    wqk = nc.declare_dram_parameter("wqk", [128, NPAIR, 2, 128], f32,
                                    isOutput=False)
    bqk = nc.declare_dram_parameter("bqk", [128, NPAIR, 2], f32,
                                    isOutput=False)
    wv2 = nc.declare_dram_parameter("wv2", [128, NPAIR, 130], bf16,
                                    isOutput=False)
    bv2 = nc.declare_dram_parameter("bv2", [1, NPAIR, 130], bf16,
                                    isOutput=False)
    out = nc.declare_dram_parameter("out", [S, D], f32, isOutput=True)

    with tile.TileContext(nc) as tc, contextlib.ExitStack() as ctx:
        singles = ctx.enter_context(tc.tile_pool(name="singles", bufs=1))
        qk_pool = ctx.enter_context(tc.tile_pool(name="qk", bufs=4))
        et_pool = ctx.enter_context(tc.tile_pool(name="et", bufs=24))
        small_sb = ctx.enter_context(tc.tile_pool(name="small_sb", bufs=4))
        # PSUM budget (8 banks): lt 2x3 + po 1 + scr 1 = 8
        lt_ps = ctx.enter_context(
            tc.tile_pool(name="lt_ps", bufs=2, space="PSUM"))
        po_ps = ctx.enter_context(
            tc.tile_pool(name="po_ps", bufs=1, space="PSUM"))
        scr_ps = ctx.enter_context(
            tc.tile_pool(name="scr_ps", bufs=1, space="PSUM"))

        # ---- persistent tiles ----
        xT_r = [singles.tile([128, S], f32r, tag=f"xT{i}", name=f"xT{i}")
                for i in range(NPAIR)]
        xT_bf = [singles.tile([128, S], bf16, tag=f"xTb{i}", name=f"xTb{i}")
                 for i in range(NPAIR)]
        v_bf = [singles.tile([128, TT, 130], bf16, tag=f"v{p}", name=f"v{p}")
                for p in range(NPAIR)]
        staging = singles.tile([128, ST, D], f32, tag="stg", name="staging")
        wqk_sb = singles.tile([128, NPAIR, 2, 128], f32r, tag="wqk",
                              name="wqk_sb")
        bqk_sb = singles.tile([128, NPAIR, 2], f32, tag="bqk", name="bqk_sb")
        wv_sb = singles.tile([128, NPAIR, 130], bf16, tag="wv", name="wv_sb")
        bv_sb = singles.tile([1, NPAIR, 130], bf16, tag="bv", name="bv_sb")
        # keep the PE busy with throwaway matmuls until real data lands, so
        # the p-state ramp (full clock after 3us of continuous work) finishes
        # before the first projection matmuls; wide moving operands keep the
        # count low (each junk matmul is paced by the PSUM write-back WAW)
        wz = singles.tile([128, 455], bf16, tag="wz", name="wz")
        nc.vector.memset(wz, 0.0)
        for _ in range(7):
            pw = po_ps.tile([128, 7, 65], f32, tag="po", name="pw")
            nc.tensor.matmul(pw.rearrange("a b c -> a (b c)"), wz[:, 0:128],
                             wz, start=True, stop=True)

        ones_bf = singles.tile([1, 128], bf16)
        nc.vector.memset(ones_bf, 1.0)
        # warm the ScalarE activation table (exp set) during the DMA lead-in
        warm = singles.tile([1, 1], f32, tag="warm", name="warm")
        nc.vector.memset(warm, 0.0)
        nc.scalar.activation(warm, warm, Exp)

        # ---- input DMAs, in lead-in-criticality order ----
        # f32r SBUF data must come from a rounding producer (DVE copy), not
        # straight DMA; stage x^T blocks through a cycling pool of f32 tiles.
        xs_pool = ctx.enter_context(tc.tile_pool(name="xs", bufs=3))
        xstage = {}

        def load_xt(p):
            xstage[p] = xs_pool.tile([128, S], f32, tag="xs", name=f"xs{p}")
            nc.sync.dma_start(out=xstage[p], in_=xt[p * 128:(p + 1) * 128, :])

        def round_xt(p, engine):
            engine.tensor_copy(xT_r[p], xstage.pop(p))

        wqk_stage = singles.tile([128, NPAIR, 2, 128], f32, tag="wqks",
                                 name="wqk_stage")
        # lead-in criticality order: x^T block-0 s-half 0 first, then the
        # pair-0 weights/bias, then the rest
        xstage[0] = xs_pool.tile([128, S], f32, tag="xs", name="xs0")
        nc.sync.dma_start(out=xstage[0][:, 0:256], in_=xt[0:128, 0:256])
        nc.sync.dma_start(out=wqk_stage[:, 0:1, :, :], in_=wqk[:, 0:1, :, :])
        nc.sync.dma_start(out=bqk_sb, in_=bqk[:, :, :])
        nc.sync.dma_start(out=xstage[0][:, 256:512], in_=xt[0:128, 256:512])
        nc.sync.dma_start(out=xstage[0][:, 512:1024], in_=xt[0:128, 512:1024])
        nc.sync.dma_start(out=wqk_stage[:, 1:NPAIR, :, :],
                          in_=wqk[:, 1:NPAIR, :, :])
        load_xt(1)
        nc.sync.dma_start(out=xT_bf[0], in_=xtb[0:128, :])
        nc.sync.dma_start(out=wv_sb, in_=wv2[:, :, :])
        nc.sync.dma_start(out=bv_sb, in_=bv2[:, :, :])
        load_xt(2)
        load_xt(3)
        load_xt(4)
        load_xt(5)
        for p in range(1, NPAIR):
            nc.sync.dma_start(out=xT_bf[p], in_=xtb[p * 128:(p + 1) * 128, :])

        qT = {}
        kT = {}

        def emit_proj0():
            """Lead-in-critical pair-0 projection: s-half-0 q/k first (the
            first exp tiles need only those), k-half-0 routed through an lt
            tile so it does not serialize behind q on the scratch bank, and
            the x^T f32r rounding split per half."""
            nc.vector.tensor_copy(xT_r[0][:, 0:256], xstage[0][:, 0:256])
            nc.vector.tensor_copy(wqk_sb[:, 0:1, :, :],
                                  wqk_stage[:, 0:1, :, :])
            qT[0] = qk_pool.tile([128, S], f32r, tag="qT", name="qT0")
            kT[0] = qk_pool.tile([128, S], f32r, tag="kT", name="kT0")
            psq0 = scr_ps.tile([128, 512], f32, tag="scr", name="psq0")
            ltk = lt_ps.tile([128, 3, 512], f32, tag="lt", name="ltk")
            # quarter-pipelined: each 256-col piece chases its x quarter
            nc.tensor.matmul(psq0[:, 0:256], wqk_sb[:, 0, 0, :],
                             xT_r[0][:, 0:256], start=True, stop=True)
            nc.tensor.matmul(ltk[:, 0, 0:256], wqk_sb[:, 0, 1, :],
                             xT_r[0][:, 0:256], start=True, stop=True)
            nc.vector.tensor_scalar_add(qT[0][:, 0:256], psq0[:, 0:256],
                                        bqk_sb[:, 0, 0:1])
            nc.vector.tensor_scalar_add(kT[0][:, 0:256], ltk[:, 0, 0:256],
                                        bqk_sb[:, 0, 1:2])
            nc.vector.tensor_copy(xT_r[0][:, 256:512], xstage[0][:, 256:512])
            nc.tensor.matmul(psq0[:, 256:512], wqk_sb[:, 0, 0, :],
                             xT_r[0][:, 256:512], start=True, stop=True)
            nc.tensor.matmul(ltk[:, 0, 256:512], wqk_sb[:, 0, 1, :],
                             xT_r[0][:, 256:512], start=True, stop=True)
            nc.vector.tensor_scalar_add(qT[0][:, 256:512], psq0[:, 256:512],
                                        bqk_sb[:, 0, 0:1])
            nc.vector.tensor_scalar_add(kT[0][:, 256:512], ltk[:, 0, 256:512],
                                        bqk_sb[:, 0, 1:2])
            nc.vector.tensor_copy(xT_r[0][:, 512:1024],
                                  xstage.pop(0)[:, 512:1024])
            psq1 = scr_ps.tile([128, 512], f32, tag="scr", name="psq1")
            nc.tensor.matmul(psq1, wqk_sb[:, 0, 0, :], xT_r[0][:, 512:1024],
                             start=True, stop=True)
            nc.vector.tensor_scalar_add(qT[0][:, 512:1024], psq1,
                                        bqk_sb[:, 0, 0:1])
            psk1 = scr_ps.tile([128, 512], f32, tag="scr", name="psk1")
            nc.tensor.matmul(psk1, wqk_sb[:, 0, 1, :], xT_r[0][:, 512:1024],
                             start=True, stop=True)
            nc.vector.tensor_scalar_add(kT[0][:, 512:1024], psk1,
                                        bqk_sb[:, 0, 1:2])
            nc.vector.tensor_copy(wqk_sb[:, 1:NPAIR, :, :],
                                  wqk_stage[:, 1:NPAIR, :, :])
            round_xt(1, nc.vector)
            for p in range(2, NPAIR):
                round_xt(p, nc.gpsimd)

        def emit_proj(p):
            """q/k projections of pair p: one blockdiag [128,512] matmul per
            (q|k, s-half) + fused bias add into f32r SBUF."""
            qT[p] = qk_pool.tile([128, S], f32r, tag="qT", name=f"qT{p}")
            kT[p] = qk_pool.tile([128, S], f32r, tag="kT", name=f"kT{p}")
            for j, dst in ((0, qT[p]), (1, kT[p])):
                for sp in range(2):
                    sl = slice(sp * 512, (sp + 1) * 512)
                    ps = scr_ps.tile([128, 512], f32, tag="scr", name="psqk")
                    nc.tensor.matmul(ps, wqk_sb[:, p, j, :], xT_r[p][:, sl],
                                     start=True, stop=True)
                    nc.vector.tensor_scalar_add(dst[:, sl], ps,
                                                bqk_sb[:, p, j:j + 1])

        def emit_v(p):
            """v~ for pair p: blockdiag Wv matmul (both heads at once) plus
            rank-1 bias/ones update; v_bf[p][:, tt, 65h:65h+65]. The psum
            tiles alternate between the scratch and po banks so the four
            accumulate+copy rounds pipeline."""
            for g in range(4):
                pool = scr_ps if g % 2 == 0 else po_ps
                pv = pool.tile([128, 2, 130], f32,
                               tag=("scr" if g % 2 == 0 else "po"), name="pv")
                for j in range(2):
                    tt = 2 * g + j
                    nc.tensor.matmul(pv[:, j, :],
                                     xT_bf[p][:, tt * 128:(tt + 1) * 128],
                                     wv_sb[:, p, :], start=True, stop=False)
                    nc.tensor.matmul(pv[:, j, :], ones_bf, bv_sb[:, p, :],
                                     start=False, stop=True)
                nc.vector.tensor_copy(v_bf[p][:, 2 * g:2 * g + 2, :], pv)

        et_tiles = [None] * NTILE

        def emit_av(p, st, pool):
            """O for both heads of pair p at s-tile st + normalize."""
            po = pool.tile([128, 7, 65], f32,
                           tag=("po" if pool is po_ps else "scr"), name="po")
            for hh in range(2):
                for tt in range(TT):
                    k, off = _et_loc(p, tt, hh, st)
                    nc.tensor.matmul(po[:, hh, :],
                                     et_tiles[k][:, off:off + 128],
                                     v_bf[p][:, tt, hh * 65:hh * 65 + 65],
                                     start=(tt == 0), stop=(tt == TT - 1))
            rc = small_sb.tile([128, 2], f32, tag="rc", name="rc")
            nc.vector.reciprocal(rc, po[:, 0:2, 64])
            nc.vector.tensor_tensor(
                out=staging[:, st, :].rearrange(
                    "a (h e) -> a h e", e=HD)[:, 2 * p:2 * p + 2, :],
                in0=po[:, 0:2, 0:HD],
                in1=rc.rearrange("a (h o) -> a h o", o=1).to_broadcast(
                    (128, 2, HD)),
                op=mybir.AluOpType.mult)

        def emit_av_head(p, hh, st, pool):
            """Single-head AV + normalize (pair-5 drain)."""
            h = 2 * p + hh
            po = pool.tile([128, 7, 65], f32,
                           tag=("po" if pool is po_ps else "scr"), name="poh")
            for tt in range(TT):
                k, off = _et_loc(p, tt, hh, st)
                nc.tensor.matmul(po[:, 0, :], et_tiles[k][:, off:off + 128],
                                 v_bf[p][:, tt, hh * 65:hh * 65 + 65],
                                 start=(tt == 0), stop=(tt == TT - 1))
            rc = small_sb.tile([128, 1], f32, tag="rc1", name="rc1")
            nc.vector.reciprocal(rc, po[:, 0, 64:65])
            nc.vector.tensor_scalar_mul(
                staging[:, st, h * HD:(h + 1) * HD], po[:, 0, 0:HD], rc)

        def emit_bulk_dma(st):
            """Writeback of columns 0:640 (pairs 0-4)."""
            nc.sync.dma_start(out=out[st * 128:(st + 1) * 128, 0:640],
                              in_=staging[:, st, 0:640])

        # ---- filler plan, keyed by exp-tile index ----
        # AV(p) is ready after tile (32p+31)//3; AV(5,*,0) after tile 58.
        plan = {k: [] for k in range(NTILE)}
        plan[0].append(lambda: emit_proj(1))
        plan[1].append(lambda: emit_v(0))
        plan[3].append(lambda: emit_proj(2))
        plan[5].append(lambda: emit_v(1))
        plan[10].append(lambda: emit_proj(3))
        for st in range(ST):
            plan[12 + st].append(lambda s=st: emit_av(0, s, po_ps))
        plan[20].append(lambda: emit_v(2))
        plan[21].append(lambda: emit_proj(4))
        plan[22].append(lambda: emit_proj(5))
        for st in range(ST):
            plan[23 + st].append(lambda s=st: emit_av(1, s, po_ps))
        plan[31].append(lambda: emit_v(3))
        for st in range(ST):
            plan[33 + st].append(lambda s=st: emit_av(2, s, po_ps))
        plan[41].append(lambda: emit_v(4))
        plan[52].append(lambda: emit_v(5))
        for st in range(ST):
            plan[44 + st].append(
                lambda s=st: emit_av(3, s, scr_ps if s % 2 else po_ps))
        for st in range(ST):
            plan[min(54 + st, 60)].append(
                lambda s=st: emit_av(4, s, scr_ps if s % 2 else po_ps))
            plan[min(54 + st, 60)].append(lambda s=st: emit_bulk_dma(s))
        for st in range(ST):
            k = 59 + min(st // 3, 2)
            plan[k].append(
                lambda s=st: emit_av_head(5, 0, s,
                                          scr_ps if s % 2 else po_ps))

        # pair-5 head-1 drain groups for s-tiles 0-3: everything except the
        # tt=7 contribution depends only on exps through tile 62, so open
        # these accumulations in-ladder (sequential per region, never
        # interleaved across regions of one bank; Tile also serializes
        # multiple open groups within one tile, so one group per tile)
        pre_tail = {}

        def alloc_tail_slot(st, pool):
            if pool is lt_ps:
                po = lt_ps.tile([128, 3, 512], f32, tag="lt", name="pot")
                return po[:, 0, 0:65]
            po = pool.tile([128, 7, 65], f32,
                           tag=("po" if pool is po_ps else "scr"), name="pot")
            return po[:, 0, :]

        def emit_pre_tail():
            for st, pool in ((0, po_ps), (1, scr_ps), (2, lt_ps),
                             (3, lt_ps)):
                ps = alloc_tail_slot(st, pool)
                pre_tail[st] = ps
                for tt in range(TT - 1):
                    k, off = _et_loc(5, tt, 1, st)
                    nc.tensor.matmul(ps, et_tiles[k][:, off:off + 128],
                                     v_bf[5][:, tt, 65:130],
                                     start=(tt == 0), stop=False)

        plan[63].append(emit_pre_tail)

        emit_proj0()

        for k in range(NTILE):
            with tc.high_priority(offset=400):
                lt = lt_ps.tile([128, 3, 512], f32, tag="lt", name="lt")
                for j in range(3):
                    p, tt, hh, half = _CHUNKS[3 * k + j]
                    rsl = slice(hh * 64, hh * 64 + 64)
                    if k == 0:
                        for q in range(2):
                            nc.tensor.matmul(
                                lt[:, j, q * 256:(q + 1) * 256],
                                kT[p][rsl, tt * 128:(tt + 1) * 128],
                                qT[p][rsl, half * 512 + q * 256:
                                      half * 512 + (q + 1) * 256],
                                start=True, stop=True)
                    else:
                        nc.tensor.matmul(
                            lt[:, j, :],
                            kT[p][rsl, tt * 128:(tt + 1) * 128],
                            qT[p][rsl, half * 512:(half + 1) * 512],
                            start=True, stop=True)
                et = et_pool.tile([128, 3 * 512], bf16, tag="et",
                                  name=f"et{k}")
                et_tiles[k] = et
                nc.scalar.activation(et, lt.rearrange("a b c -> a (b c)"),
                                     Exp)
            for f in plan[k]:
                f()

        # ---- tail: pair-5 head-1 AV drain + pair-5 column writeback ----
        # ScalarE is idle once the exps finish: it takes over the normalize
        # multiplies (activation Copy with a per-partition scale) while DVE
        # only does the reciprocals; the freed lt banks give a 4-deep psum
        # rotation so the 8 drain groups pipeline.
        # pair-5 head-0 columns are final once the in-ladder AV(5,*,0)
        # groups drain; one early DMA halves the post-span writeback
        nc.sync.dma_start(
            out=out[:, 640:704].rearrange("(st q) e -> q st e", q=128),
            in_=staging[:, :, 640:704])

        Copy = mybir.ActivationFunctionType.Copy
        tail_pool = {4: po_ps, 5: scr_ps, 6: lt_ps, 7: lt_ps}
        # finish the four pre-opened groups with batched recips so the DVE
        # chain is not interleaved with the multiplies
        rcs = {}
        for st in range(4):
            ps = pre_tail[st]
            k, off = _et_loc(5, TT - 1, 1, st)
            nc.tensor.matmul(ps, et_tiles[k][:, off:off + 128],
                             v_bf[5][:, TT - 1, 65:130],
                             start=False, stop=True)
        for st in range(4):
            rc = small_sb.tile([128, 1], f32, tag="rc1", name="rc1")
            nc.vector.reciprocal(rc, pre_tail[st][:, 64:65])
            rcs[st] = rc
        for st in range(ST):
            if st in pre_tail:
                ps, rc = pre_tail[st], rcs[st]
            else:
                ps = alloc_tail_slot(st, tail_pool[st])
                for tt in range(TT):
                    k, off = _et_loc(5, tt, 1, st)
                    nc.tensor.matmul(ps, et_tiles[k][:, off:off + 128],
                                     v_bf[5][:, tt, 65:130],
                                     start=(tt == 0), stop=(tt == TT - 1))
                rc = small_sb.tile([128, 1], f32, tag="rc1", name="rc1")
                nc.vector.reciprocal(rc, ps[:, 64:65])
            if st % 2 == 0:
                nc.vector.tensor_scalar_mul(staging[:, st, 704:768],
                                            ps[:, 0:64], rc)
            else:
                nc.scalar.activation(staging[:, st, 704:768], ps[:, 0:64],
                                     Copy, scale=rc)
            if st == 3:
                nc.sync.dma_start(
                    out=out[0:512, 704:768].rearrange(
                        "(st q) e -> q st e", q=128),
                    in_=staging[:, 0:4, 704:768])
            elif st == 6:
                nc.sync.dma_start(
                    out=out[512:896, 704:768].rearrange(
                        "(st q) e -> q st e", q=128),
                    in_=staging[:, 4:7, 704:768])
            elif st == 7:
                nc.sync.dma_start(out=out[896:1024, 704:768],
                                  in_=staging[:, 7, 704:768])

    nc.compile()
    return nc


def _get_nc():
    if "nc" not in _CACHE:
        _CACHE["nc"] = _build()
    return _CACHE["nc"]


def _prep_inputs(inputs):
    """Host-side packing (numpy; layout only, no model FLOPs)."""
    import ml_dtypes

    bf16 = ml_dtypes.bfloat16
    seq = np.ascontiguousarray(np.asarray(inputs["sequences"],
                                          dtype=np.float32))
    Wq = np.asarray(inputs["Wq"], dtype=np.float32)
    Wk = np.asarray(inputs["Wk"], dtype=np.float32)
    Wv = np.asarray(inputs["Wv"], dtype=np.float32)
    bq = np.asarray(inputs["bq"], dtype=np.float32)
    bk = np.asarray(inputs["bk"], dtype=np.float32)
    bv = np.asarray(inputs["bv"], dtype=np.float32)

    wqk = np.zeros((128, NPAIR, 2, 128), dtype=np.float32)
    bqk = np.zeros((128, NPAIR, 2), dtype=np.float32)
    for p in range(NPAIR):
        for j, (W, b) in enumerate(((Wq, bq), (Wk, bk))):
            wqk[0:64, p, j, 0:64] = W[2 * p]
            wqk[64:128, p, j, 64:128] = W[2 * p + 1]
            bqk[0:64, p, j] = b[2 * p]
            bqk[64:128, p, j] = b[2 * p + 1]
    wv2 = np.zeros((128, NPAIR, 130), dtype=np.float32)
    bv2 = np.zeros((1, NPAIR, 130), dtype=np.float32)
    for p in range(NPAIR):
        wv2[0:64, p, 0:64] = Wv[2 * p]
        wv2[64:128, p, 65:129] = Wv[2 * p + 1]
        bv2[0, p, 0:64] = bv[2 * p]
        bv2[0, p, 64] = 1.0
        bv2[0, p, 65:129] = bv[2 * p + 1]
        bv2[0, p, 129] = 1.0
    common = {
        "wqk": wqk,
        "bqk": bqk,
        "wv2": wv2.astype(bf16),
        "bv2": bv2.astype(bf16),
    }
    in_maps = []
    for b in range(NCORES):
        xt = np.ascontiguousarray(seq[b].T)
        in_maps.append(dict(common, xt=xt, xtb=xt.astype(bf16)))
    return in_maps


def kernel(**inputs) -> np.ndarray:
    from concourse.bass_utils import run_bass_kernel_spmd

    nc = _get_nc()
    in_maps = _prep_inputs(inputs)
    res = run_bass_kernel_spmd(nc, in_maps, list(range(NCORES)))
    return np.stack([res.results[b]["out"] for b in range(NCORES)], axis=0)


# revision 66
# speedup vs baseline: 1.0044x; 1.0044x over previous
"""Multi-head attention Trainium2 Bass kernel.

Problem: B=8, S=1024, D=768, H=12, head_dim=64; per-head block-diagonal QKV
projections + softmax attention (no 1/sqrt(hd) scaling).

Sharding: data-parallel over batch - one batch element per NeuronCore (8
cores). No collectives; host scatters inputs / gathers outputs.

Host-side prep (free - not on the HW clock): x is pre-transposed to
xT [D, S] (f32 and bf16 copies), and the per-head weight stacks are packed
into block-diagonal pair matrices so each head-pair's QKV projection is a
single 128-contraction matmul:
  wqk [128, 6, 2, 128]  blockdiag(W[2p], W[2p+1]) for q (j=0) / k (j=1)
  bqk [128, 6, 2]       per-partition bias columns
  wv2 [128, 6, 130]     blockdiag Wv pair, 65-wide halves; col 64/129 = 0
  bv2 [1, 6, 130]       bv pair with 1.0 in cols 64/129 (softmax denom trick)

Per-core dataflow (channel-on-partition layouts; head pairs p = (2p, 2p+1)
match 128-row blocks of xT):
  qT/kT[p] = wqk[p].T @ xT[p] + bqk   (f32r, one [128,512] matmul per half)
  v~[p]    = xT_bf[p].T @ wv2[p] (+ rank-1 bv2)   [t, tt, 130] bf16
  LT       = K Q^T  in 64 psum tiles [128, 3, 512] (3 banks each, bufs=2)
  E^T      = exp(LT)  one ScalarE op per 1536-wide tile (the bottleneck:
             64 x (1536+222) cycles at 1.2 GHz ~= 94 us busy)
  O        = E @ v~ per (pair, s-tile): [128, 2, 65]-shaped psum groups;
             col 64 of each head half = softmax denominator (ones column)
  out      = O * recip(denom) -> staging [128, 8, 768] -> chunked DMAs

Scheduling: ScalarE exp is a solid 64-tile ladder; everything else rides it
as fillers keyed by exp-tile index (see the `plan` dict): projections and
v-projections sit in AV-free tile windows, AV groups follow their pair's
last exp, and writeback DMAs are chunked per s-tile column block. PSUM
banks: lt 2x3 + po 1 + scr 1 = 8; sequential-only accumulation groups per
bank region (interleaved groups in one bank corrupt - hw start-flag
semantics). The lead-in hides the PE p-state ramp behind junk matmuls and
orders DMAs/bias-adds so the first exp fires at ~6.6 us; the tail drains
pair-5 head-1 with a 4-deep psum rotation, ScalarE-assisted normalizes,
and small trailing DMAs.
"""
import numpy as np

S = 1024
D = 768
H = 12
HD = 64
NPAIR = H // 2   # 6
NCORES = 8
ST = S // 128    # 8 s-tiles
TT = S // 128    # 8 t-tiles
NCHUNK = 192     # 512-col logit chunks
NTILE = 64       # exp tiles (3 chunks each)

_CACHE = {}


def _chunk_order():
    """Flat emission order of 512-col logit chunks: pairs sequential. Pair 0
    is grouped by (kT s-half = tt//4, qT s-half) so the first exp tiles only
    need the first q/k bias-add of the projection; middle pairs are tt-major;
    the last pair is hh-major so head-0's AV can drain while head-1's exps
    still run."""
    order = []
    for khalf in (0, 1):
        for half in (0, 1):
            for tt in range(4 * khalf, 4 * khalf + 4):
                for hh in range(2):
                    order.append((0, tt, hh, half))
    for p in range(1, NPAIR - 1):
        for tt in range(TT):
            for hh in range(2):
                order.append((p, tt, hh, 0))
                order.append((p, tt, hh, 1))
    p = NPAIR - 1
    for hh in range(2):
        for tt in range(TT):
            order.append((p, tt, hh, 0))
            order.append((p, tt, hh, 1))
    return order


_CHUNKS = _chunk_order()
_CHUNK_IDX = {key: c for c, key in enumerate(_CHUNKS)}


def _et_loc(p, tt, hh, st):
    """(exp-tile index, col offset) of the [128,128] E^T slice for s-tile st
    of unit (p, tt, hh)."""
    c = _CHUNK_IDX[(p, tt, hh, 0 if st < 4 else 1)]
    return c // 3, (c % 3) * 512 + (st % 4) * 128


def _build():
    import contextlib
    import concourse.bacc as bacc
    import concourse.mybir as mybir
    import concourse.tile as tile

    f32 = mybir.dt.float32
    f32r = mybir.dt.float32r
    bf16 = mybir.dt.bfloat16
    Exp = mybir.ActivationFunctionType.Exp

    nc = bacc.Bacc("TRN2", target_bir_lowering=False, debug=False,
                   num_devices=NCORES)
    xt = nc.declare_dram_parameter("xt", [D, S], f32, isOutput=False)
    xtb = nc.declare_dram_parameter("xtb", [D, S], bf16, isOutput=False)
    wqk = nc.declare_dram_parameter("wqk", [128, NPAIR, 2, 128], f32,
                                    isOutput=False)
    bqk = nc.declare_dram_parameter("bqk", [128, NPAIR, 2], f32,
                                    isOutput=False)
    wv2 = nc.declare_dram_parameter("wv2", [128, NPAIR, 130], bf16,
                                    isOutput=False)
    bv2 = nc.declare_dram_parameter("bv2", [1, NPAIR, 130], bf16,
                                    isOutput=False)
    out = nc.declare_dram_parameter("out", [S, D], f32, isOutput=True)

    with tile.TileContext(nc) as tc, contextlib.ExitStack() as ctx:
        singles = ctx.enter_context(tc.tile_pool(name="singles", bufs=1))
        qk_pool = ctx.enter_context(tc.tile_pool(name="qk", bufs=4))
        et_pool = ctx.enter_context(tc.tile_pool(name="et", bufs=24))
        small_sb = ctx.enter_context(tc.tile_pool(name="small_sb", bufs=4))
        # PSUM budget (8 banks): lt 2x3 + po 1 + scr 1 = 8
        lt_ps = ctx.enter_context(
            tc.tile_pool(name="lt_ps", bufs=2, space="PSUM"))
        po_ps = ctx.enter_context(
            tc.tile_pool(name="po_ps", bufs=1, space="PSUM"))
        scr_ps = ctx.enter_context(
            tc.tile_pool(name="scr_ps", bufs=1, space="PSUM"))

        # ---- persistent tiles ----
        xT_r = [singles.tile([128, S], f32r, tag=f"xT{i}", name=f"xT{i}")
                for i in range(NPAIR)]
        xT_bf = [singles.tile([128, S], bf16, tag=f"xTb{i}", name=f"xTb{i}")
                 for i in range(NPAIR)]
        v_bf = [singles.tile([128, TT, 130], bf16, tag=f"v{p}", name=f"v{p}")
                for p in range(NPAIR)]
        staging = singles.tile([128, ST, D], f32, tag="stg", name="staging")
        wqk_sb = singles.tile([128, NPAIR, 2, 128], f32r, tag="wqk",
                              name="wqk_sb")
        bqk_sb = singles.tile([128, NPAIR, 2], f32, tag="bqk", name="bqk_sb")
        wv_sb = singles.tile([128, NPAIR, 130], bf16, tag="wv", name="wv_sb")
        bv_sb = singles.tile([1, NPAIR, 130], bf16, tag="bv", name="bv_sb")
        # keep the PE busy with throwaway matmuls until real data lands, so
        # the p-state ramp (full clock after 3us of continuous work) finishes
        # before the first projection matmuls; wide moving operands keep the
        # count low (each junk matmul is paced by the PSUM write-back WAW)
        wz = singles.tile([128, 455], bf16, tag="wz", name="wz")
        nc.vector.memset(wz, 0.0)
        for _ in range(7):
            pw = po_ps.tile([128, 7, 65], f32, tag="po", name="pw")
            nc.tensor.matmul(pw.rearrange("a b c -> a (b c)"), wz[:, 0:128],
                             wz, start=True, stop=True)

        ones_bf = singles.tile([1, 128], bf16)
        nc.vector.memset(ones_bf, 1.0)
        # warm the ScalarE activation table (exp set) during the DMA lead-in
        warm = singles.tile([1, 1], f32, tag="warm", name="warm")
        nc.vector.memset(warm, 0.0)
        nc.scalar.activation(warm, warm, Exp)

        # ---- input DMAs, in lead-in-criticality order ----
        # f32r SBUF data must come from a rounding producer (DVE copy), not
        # straight DMA; stage x^T blocks through a cycling pool of f32 tiles.
        xs_pool = ctx.enter_context(tc.tile_pool(name="xs", bufs=3))
        xstage = {}

        def load_xt(p):
            xstage[p] = xs_pool.tile([128, S], f32, tag="xs", name=f"xs{p}")
            nc.sync.dma_start(out=xstage[p], in_=xt[p * 128:(p + 1) * 128, :])

        def round_xt(p, engine):
            engine.tensor_copy(xT_r[p], xstage.pop(p))

        wqk_stage = singles.tile([128, NPAIR, 2, 128], f32, tag="wqks",
                                 name="wqk_stage")
        # lead-in criticality order: x^T block-0 s-half 0 first, then the
        # pair-0 weights/bias, then the rest
        xstage[0] = xs_pool.tile([128, S], f32, tag="xs", name="xs0")
        nc.sync.dma_start(out=xstage[0][:, 0:512], in_=xt[0:128, 0:512])
        nc.sync.dma_start(out=wqk_stage[:, 0:1, :, :], in_=wqk[:, 0:1, :, :])
        nc.sync.dma_start(out=bqk_sb, in_=bqk[:, :, :])
        nc.sync.dma_start(out=xstage[0][:, 512:1024], in_=xt[0:128, 512:1024])
        nc.sync.dma_start(out=wqk_stage[:, 1:NPAIR, :, :],
                          in_=wqk[:, 1:NPAIR, :, :])
        load_xt(1)
        nc.sync.dma_start(out=xT_bf[0], in_=xtb[0:128, :])
        nc.sync.dma_start(out=wv_sb, in_=wv2[:, :, :])
        nc.sync.dma_start(out=bv_sb, in_=bv2[:, :, :])
        load_xt(2)
        load_xt(3)
        load_xt(4)
        load_xt(5)
        for p in range(1, NPAIR):
            nc.sync.dma_start(out=xT_bf[p], in_=xtb[p * 128:(p + 1) * 128, :])

        qT = {}
        kT = {}

        def emit_proj0():
            """Lead-in-critical pair-0 projection: s-half-0 q/k first (the
            first exp tiles need only those), k-half-0 routed through an lt
            tile so it does not serialize behind q on the scratch bank, and
            the x^T f32r rounding split per half."""
            nc.vector.tensor_copy(xT_r[0][:, 0:512], xstage[0][:, 0:512])
            nc.vector.tensor_copy(wqk_sb[:, 0:1, :, :],
                                  wqk_stage[:, 0:1, :, :])
            qT[0] = qk_pool.tile([128, S], f32r, tag="qT", name="qT0")
            kT[0] = qk_pool.tile([128, S], f32r, tag="kT", name="kT0")
            psq0 = scr_ps.tile([128, 512], f32, tag="scr", name="psq0")
            nc.tensor.matmul(psq0, wqk_sb[:, 0, 0, :], xT_r[0][:, 0:512],
                             start=True, stop=True)
            ltk = lt_ps.tile([128, 3, 512], f32, tag="lt", name="ltk")
            nc.tensor.matmul(ltk[:, 0, :], wqk_sb[:, 0, 1, :],
                             xT_r[0][:, 0:512], start=True, stop=True)
            nc.vector.tensor_scalar_add(qT[0][:, 0:512], psq0,
                                        bqk_sb[:, 0, 0:1])
            # split: the first exp tile only needs kT cols 0:256
            nc.vector.tensor_scalar_add(kT[0][:, 0:256], ltk[:, 0, 0:256],
                                        bqk_sb[:, 0, 1:2])
            nc.vector.tensor_scalar_add(kT[0][:, 256:512], ltk[:, 0, 256:512],
                                        bqk_sb[:, 0, 1:2])
            nc.vector.tensor_copy(xT_r[0][:, 512:1024],
                                  xstage.pop(0)[:, 512:1024])
            psq1 = scr_ps.tile([128, 512], f32, tag="scr", name="psq1")
            nc.tensor.matmul(psq1, wqk_sb[:, 0, 0, :], xT_r[0][:, 512:1024],
                             start=True, stop=True)
            nc.vector.tensor_scalar_add(qT[0][:, 512:1024], psq1,
                                        bqk_sb[:, 0, 0:1])
            psk1 = scr_ps.tile([128, 512], f32, tag="scr", name="psk1")
            nc.tensor.matmul(psk1, wqk_sb[:, 0, 1, :], xT_r[0][:, 512:1024],
                             start=True, stop=True)
            nc.vector.tensor_scalar_add(kT[0][:, 512:1024], psk1,
                                        bqk_sb[:, 0, 1:2])
            nc.vector.tensor_copy(wqk_sb[:, 1:NPAIR, :, :],
                                  wqk_stage[:, 1:NPAIR, :, :])
            round_xt(1, nc.vector)
            for p in range(2, NPAIR):
                round_xt(p, nc.gpsimd)

        def emit_proj(p):
            """q/k projections of pair p: one blockdiag [128,512] matmul per
            (q|k, s-half) + fused bias add into f32r SBUF."""
            qT[p] = qk_pool.tile([128, S], f32r, tag="qT", name=f"qT{p}")
            kT[p] = qk_pool.tile([128, S], f32r, tag="kT", name=f"kT{p}")
            for j, dst in ((0, qT[p]), (1, kT[p])):
                for sp in range(2):
                    sl = slice(sp * 512, (sp + 1) * 512)
                    ps = scr_ps.tile([128, 512], f32, tag="scr", name="psqk")
                    nc.tensor.matmul(ps, wqk_sb[:, p, j, :], xT_r[p][:, sl],
                                     start=True, stop=True)
                    nc.vector.tensor_scalar_add(dst[:, sl], ps,
                                                bqk_sb[:, p, j:j + 1])

        def emit_v(p):
            """v~ for pair p: blockdiag Wv matmul (both heads at once) plus
            rank-1 bias/ones update; v_bf[p][:, tt, 65h:65h+65]. The psum
            tiles alternate between the scratch and po banks so the four
            accumulate+copy rounds pipeline."""
            for g in range(4):
                pool = scr_ps if g % 2 == 0 else po_ps
                pv = pool.tile([128, 2, 130], f32,
                               tag=("scr" if g % 2 == 0 else "po"), name="pv")
                for j in range(2):
                    tt = 2 * g + j
                    nc.tensor.matmul(pv[:, j, :],
                                     xT_bf[p][:, tt * 128:(tt + 1) * 128],
                                     wv_sb[:, p, :], start=True, stop=False)
                    nc.tensor.matmul(pv[:, j, :], ones_bf, bv_sb[:, p, :],
                                     start=False, stop=True)
                nc.vector.tensor_copy(v_bf[p][:, 2 * g:2 * g + 2, :], pv)

        et_tiles = [None] * NTILE

        def emit_av(p, st, pool):
            """O for both heads of pair p at s-tile st + normalize."""
            po = pool.tile([128, 7, 65], f32,
                           tag=("po" if pool is po_ps else "scr"), name="po")
            for hh in range(2):
                for tt in range(TT):
                    k, off = _et_loc(p, tt, hh, st)
                    nc.tensor.matmul(po[:, hh, :],
                                     et_tiles[k][:, off:off + 128],
                                     v_bf[p][:, tt, hh * 65:hh * 65 + 65],
                                     start=(tt == 0), stop=(tt == TT - 1))
            rc = small_sb.tile([128, 2], f32, tag="rc", name="rc")
            nc.vector.reciprocal(rc, po[:, 0:2, 64])
            nc.vector.tensor_tensor(
                out=staging[:, st, :].rearrange(
                    "a (h e) -> a h e", e=HD)[:, 2 * p:2 * p + 2, :],
                in0=po[:, 0:2, 0:HD],
                in1=rc.rearrange("a (h o) -> a h o", o=1).to_broadcast(
                    (128, 2, HD)),
                op=mybir.AluOpType.mult)

        def emit_av_head(p, hh, st, pool):
            """Single-head AV + normalize (pair-5 drain)."""
            h = 2 * p + hh
            po = pool.tile([128, 7, 65], f32,
                           tag=("po" if pool is po_ps else "scr"), name="poh")
            for tt in range(TT):
                k, off = _et_loc(p, tt, hh, st)
                nc.tensor.matmul(po[:, 0, :], et_tiles[k][:, off:off + 128],
                                 v_bf[p][:, tt, hh * 65:hh * 65 + 65],
                                 start=(tt == 0), stop=(tt == TT - 1))
            rc = small_sb.tile([128, 1], f32, tag="rc1", name="rc1")
            nc.vector.reciprocal(rc, po[:, 0, 64:65])
            nc.vector.tensor_scalar_mul(
                staging[:, st, h * HD:(h + 1) * HD], po[:, 0, 0:HD], rc)

        def emit_bulk_dma(st):
            """Writeback of columns 0:640 (pairs 0-4)."""
            nc.sync.dma_start(out=out[st * 128:(st + 1) * 128, 0:640],
                              in_=staging[:, st, 0:640])

        # ---- filler plan, keyed by exp-tile index ----
        # AV(p) is ready after tile (32p+31)//3; AV(5,*,0) after tile 58.
        plan = {k: [] for k in range(NTILE)}
        plan[0].append(lambda: emit_proj(1))
        plan[1].append(lambda: emit_v(0))
        plan[3].append(lambda: emit_proj(2))
        plan[5].append(lambda: emit_v(1))
        plan[10].append(lambda: emit_proj(3))
        for st in range(ST):
            plan[12 + st].append(lambda s=st: emit_av(0, s, po_ps))
        plan[20].append(lambda: emit_v(2))
        plan[21].append(lambda: emit_proj(4))
        plan[22].append(lambda: emit_proj(5))
        for st in range(ST):
            plan[23 + st].append(lambda s=st: emit_av(1, s, po_ps))
        plan[31].append(lambda: emit_v(3))
        for st in range(ST):
            plan[33 + st].append(lambda s=st: emit_av(2, s, po_ps))
        plan[41].append(lambda: emit_v(4))
        plan[52].append(lambda: emit_v(5))
        for st in range(ST):
            plan[44 + st].append(
                lambda s=st: emit_av(3, s, scr_ps if s % 2 else po_ps))
        for st in range(ST):
            plan[min(54 + st, 60)].append(
                lambda s=st: emit_av(4, s, scr_ps if s % 2 else po_ps))
            plan[min(54 + st, 60)].append(lambda s=st: emit_bulk_dma(s))
        for st in range(ST):
            k = 59 + min(st // 3, 2)
            plan[k].append(
                lambda s=st: emit_av_head(5, 0, s,
                                          scr_ps if s % 2 else po_ps))

        # pair-5 head-1 drain groups for s-tiles 0-3: everything except the
        # tt=7 contribution depends only on exps through tile 62, so open
        # these accumulations in-ladder (sequential per region, never
        # interleaved across regions of one bank; Tile also serializes
        # multiple open groups within one tile, so one group per tile)
        pre_tail = {}

        def alloc_tail_slot(st, pool):
            if pool is lt_ps:
                po = lt_ps.tile([128, 3, 512], f32, tag="lt", name="pot")
                return po[:, 0, 0:65]
            po = pool.tile([128, 7, 65], f32,
                           tag=("po" if pool is po_ps else "scr"), name="pot")
            return po[:, 0, :]

        def emit_pre_tail():
            for st, pool in ((0, po_ps), (1, scr_ps), (2, lt_ps),
                             (3, lt_ps)):
                ps = alloc_tail_slot(st, pool)
                pre_tail[st] = ps
                for tt in range(TT - 1):
                    k, off = _et_loc(5, tt, 1, st)
                    nc.tensor.matmul(ps, et_tiles[k][:, off:off + 128],
                                     v_bf[5][:, tt, 65:130],
                                     start=(tt == 0), stop=False)

        plan[63].append(emit_pre_tail)

        emit_proj0()

        for k in range(NTILE):
            with tc.high_priority(offset=400):
                lt = lt_ps.tile([128, 3, 512], f32, tag="lt", name="lt")
                for j in range(3):
                    p, tt, hh, half = _CHUNKS[3 * k + j]
                    rsl = slice(hh * 64, hh * 64 + 64)
                    nc.tensor.matmul(
                        lt[:, j, :],
                        kT[p][rsl, tt * 128:(tt + 1) * 128],
                        qT[p][rsl, half * 512:(half + 1) * 512],
                        start=True, stop=True)
                et = et_pool.tile([128, 3 * 512], bf16, tag="et",
                                  name=f"et{k}")
                et_tiles[k] = et
                nc.scalar.activation(et, lt.rearrange("a b c -> a (b c)"),
                                     Exp)
            for f in plan[k]:
                f()

        # ---- tail: pair-5 head-1 AV drain + pair-5 column writeback ----
        # ScalarE is idle once the exps finish: it takes over the normalize
        # multiplies (activation Copy with a per-partition scale) while DVE
        # only does the reciprocals; the freed lt banks give a 4-deep psum
        # rotation so the 8 drain groups pipeline.
        # pair-5 head-0 columns are final once the in-ladder AV(5,*,0)
        # groups drain; one early DMA halves the post-span writeback
        nc.sync.dma_start(
            out=out[:, 640:704].rearrange("(st q) e -> q st e", q=128),
            in_=staging[:, :, 640:704])

        Copy = mybir.ActivationFunctionType.Copy
        tail_pool = {4: po_ps, 5: scr_ps, 6: lt_ps, 7: lt_ps}
        # finish the four pre-opened groups with batched recips so the DVE
        # chain is not interleaved with the multiplies
        rcs = {}
        for st in range(4):
            ps = pre_tail[st]
            k, off = _et_loc(5, TT - 1, 1, st)
            nc.tensor.matmul(ps, et_tiles[k][:, off:off + 128],
                             v_bf[5][:, TT - 1, 65:130],
                             start=False, stop=True)
        for st in range(4):
            rc = small_sb.tile([128, 1], f32, tag="rc1", name="rc1")
            nc.vector.reciprocal(rc, pre_tail[st][:, 64:65])
            rcs[st] = rc
        for st in range(ST):
            if st in pre_tail:
                ps, rc = pre_tail[st], rcs[st]
            else:
                ps = alloc_tail_slot(st, tail_pool[st])
                for tt in range(TT):
                    k, off = _et_loc(5, tt, 1, st)
                    nc.tensor.matmul(ps, et_tiles[k][:, off:off + 128],
                                     v_bf[5][:, tt, 65:130],
                                     start=(tt == 0), stop=(tt == TT - 1))
                rc = small_sb.tile([128, 1], f32, tag="rc1", name="rc1")
                nc.vector.reciprocal(rc, ps[:, 64:65])
            if st % 2 == 0:
                nc.vector.tensor_scalar_mul(staging[:, st, 704:768],
                                            ps[:, 0:64], rc)
            else:
                nc.scalar.activation(staging[:, st, 704:768], ps[:, 0:64],
                                     Copy, scale=rc)
            if st == 3:
                nc.sync.dma_start(
                    out=out[0:512, 704:768].rearrange(
                        "(st q) e -> q st e", q=128),
                    in_=staging[:, 0:4, 704:768])
            elif st == 6:
                nc.sync.dma_start(
                    out=out[512:896, 704:768].rearrange(
                        "(st q) e -> q st e", q=128),
                    in_=staging[:, 4:7, 704:768])
            elif st == 7:
                nc.sync.dma_start(out=out[896:1024, 704:768],
                                  in_=staging[:, 7, 704:768])

    nc.compile()
    return nc


def _get_nc():
    if "nc" not in _CACHE:
        _CACHE["nc"] = _build()
    return _CACHE["nc"]


def _prep_inputs(inputs):
    """Host-side packing (numpy; layout only, no model FLOPs)."""
    import ml_dtypes

    bf16 = ml_dtypes.bfloat16
    seq = np.ascontiguousarray(np.asarray(inputs["sequences"],
                                          dtype=np.float32))
    Wq = np.asarray(inputs["Wq"], dtype=np.float32)
    Wk = np.asarray(inputs["Wk"], dtype=np.float32)
    Wv = np.asarray(inputs["Wv"], dtype=np.float32)
    bq = np.asarray(inputs["bq"], dtype=np.float32)
    bk = np.asarray(inputs["bk"], dtype=np.float32)
    bv = np.asarray(inputs["bv"], dtype=np.float32)

    wqk = np.zeros((128, NPAIR, 2, 128), dtype=np.float32)
    bqk = np.zeros((128, NPAIR, 2), dtype=np.float32)
    for p in range(NPAIR):
        for j, (W, b) in enumerate(((Wq, bq), (Wk, bk))):
            wqk[0:64, p, j, 0:64] = W[2 * p]
            wqk[64:128, p, j, 64:128] = W[2 * p + 1]
            bqk[0:64, p, j] = b[2 * p]
            bqk[64:128, p, j] = b[2 * p + 1]
    wv2 = np.zeros((128, NPAIR, 130), dtype=np.float32)
    bv2 = np.zeros((1, NPAIR, 130), dtype=np.float32)
    for p in range(NPAIR):
        wv2[0:64, p, 0:64] = Wv[2 * p]
        wv2[64:128, p, 65:129] = Wv[2 * p + 1]
        bv2[0, p, 0:64] = bv[2 * p]
        bv2[0, p, 64] = 1.0
        bv2[0, p, 65:129] = bv[2 * p + 1]
        bv2[0, p, 129] = 1.0
    common = {
        "wqk": wqk,
        "bqk": bqk,
        "wv2": wv2.astype(bf16),
        "bv2": bv2.astype(bf16),
    }
    in_maps = []
    for b in range(NCORES):
        xt = np.ascontiguousarray(seq[b].T)
        in_maps.append(dict(common, xt=xt, xtb=xt.astype(bf16)))
    return in_maps


def kernel(**inputs) -> np.ndarray:
    from concourse.bass_utils import run_bass_kernel_spmd

    nc = _get_nc()
    in_maps = _prep_inputs(inputs)
    res = run_bass_kernel_spmd(nc, in_maps, list(range(NCORES)))
    return np.stack([res.results[b]["out"] for b in range(NCORES)], axis=0)
